# revision 30
# baseline (speedup 1.0000x reference)
"""NURBS surface evaluation on 8 Trainium2 NeuronCores.

Math: the reference computes, for output grid point (e, f) and channel d,
    surf[e, f, d] = sum_{l,r} bx[e,l] * by[f,r] * P[ix[l,e], iy[r,f], d]
which factorizes exactly as a pair of matmuls against sparse basis-scatter
matrices built on the host from the (tiny) knot vectors:
    BxD[e, ix[l,e]] = bx[e,l]      (1024, 256)
    ByD[iy[r,f], f] = by[f,r]      (256, 1024)
    surf[:, :, d]   = BxD @ P[:, :, d] @ ByD
The Cox-de Boor prep is O(10^4) flops on 520 knot values -- done on host
(bit-matching the jax reference); the 2 GFLOP / 12.6 MB contraction runs
on device.

Sharding: the e (Ex) axis is split across 8 cores (128 rows each) -- fully
data parallel, no collectives. Each core computes, in fp32:
    T1Tw[b][r, e_loc] = sum_i Pext[i, w_b + r, d] * BxT[i, e_loc] (stage 1)
    out_d[e_loc, f]   = sum_r T1Tw[b][r, e_loc] * ByD'[w_b + r, f] (stage 2)
Stage 1 contracts only over the core's control-row footprint window (x
spans are monotone in e, so ~40 of 256 rows; padded to K=128). Stage 2 is
banded in unwrapped span space j' = span_y + l (in [0, 262)): each f-block
of 256 touches one j' window of width <= ~80 <= 128, so every output
column is streamed through the PE exactly once (a dense K=256 contraction
would stream everything twice). Inputs degenerate to a dense fallback path
if the windows exceed 128.

Perf notes (measured): fp32 matmul streams at 4 cycles/col (2 half-speed
passes); a bf16 warm-up matmul stream opens the HAM clock gate (1.2 ->
2.4 GHz) while the input DMAs are in flight; Tile's entry/exit all-engine
barriers are stripped (waits on the output-DMA completion semaphores are
kept); instructions carrying >1 semaphore wait are split into single-wait
NoOps for this container's walrus. Per-core traffic: ~0.85 MB in,
1.5 MB out, ~25 us/NEFF of which ~8 us is fixed bootstrap/teardown.
"""

import os

import numpy as np

DEGREE = 3
OUT_XY = 1024
N_CTRL = 256
EPS = 1e-05
N_CORES = 8
EC = OUT_XY // N_CORES  # output rows per core

# Stage-2 matmuls in float32r (1 cycle/row vs 4 for fp32, ~tf32 accuracy,
# norm-rel err ~1.5e-4 vs ~1e-7 exact). Off by default: exactness first.
F32R = bool(int(os.environ.get("NURBS_F32R", "0")))

# Set by kernel() on each call: BassKernelResults of the last device run
# (test harnessing only; carries exec_time_ns when tracing is enabled).
last_results = None


# ----------------------------------------------------------------------------
# Host-side prep: knots, spans, Cox-de Boor basis, scatter matrices
# ----------------------------------------------------------------------------

def _normalize_knots_np(k):
    k = np.where(k < 0.0, np.float32(1e-4), k.astype(np.float32))
    k = np.cumsum(k, dtype=np.float32)
    return ((k - k[0]) / (k[-1] - k[0])).astype(np.float32)


def _prep_scalars(knot_x_row, knot_y_row):
    """Normalized knot vectors and the eval grid.

    cumsum/linspace rounding depends on the backend; run these two tiny ops
    through jax-on-CPU when available so the values match the jax reference
    bit-for-bit. Everything downstream (searchsorted, basis arithmetic) is
    elementwise IEEE fp32 and matches numpy exactly.
    """
    try:
        import jax
        import jax.numpy as jnp

        cpu = jax.devices("cpu")[0]
        with jax.default_device(cpu):
            def nk(k):
                k = jnp.where(k < 0.0, jnp.asarray(1e-4, k.dtype), k)
                k = jnp.cumsum(k)
                return (k - k[0]) / (k[-1] - k[0])

            kx = np.asarray(nk(jnp.asarray(knot_x_row)))
            ky = np.asarray(nk(jnp.asarray(knot_y_row)))
            ev = np.asarray(jnp.linspace(EPS, 1.0 - EPS, OUT_XY, dtype=jnp.float32))
        return (kx.astype(np.float32), ky.astype(np.float32),
                ev.astype(np.float32))
    except Exception:
        ev = np.linspace(EPS, 1.0 - EPS, OUT_XY).astype(np.float32)
        return _normalize_knots_np(knot_x_row), _normalize_knots_np(knot_y_row), ev


def _find_spans(u, knots):
    spans = np.searchsorted(knots, u, side="right") - 1
    return np.where(u == knots[N_CTRL], N_CTRL - 1, spans)


def _basis(u, knots, span):
    # Cox-de Boor recursion, literal port of the reference (fp32 throughout).
    K = knots.shape[0]
    cols = [np.ones_like(u)]
    left = [None]
    right = [None]
    for j in range(1, DEGREE + 1):
        left.append(u - knots[np.mod(span + 1 - j, K)])
        right.append(knots[np.mod(span + j, K)] - u)
        saved = np.zeros_like(u)
        new_cols = []
        for r in range(j):
            temp = cols[r] / (right[r + 1] + left[j - r])
            new_cols.append(saved + right[r + 1] * temp)
            saved = left[j - r] * temp
        new_cols.append(saved)
        cols = new_cols
    return np.stack(cols, axis=-1)  # (E, DEGREE+1)


def _host_pack(control_points, knot_vector_x, knot_vector_y):
    P = np.ascontiguousarray(np.asarray(control_points, dtype=np.float32))
    kx, ky, ev = _prep_scalars(np.asarray(knot_vector_x, np.float32)[0],
                               np.asarray(knot_vector_y, np.float32)[0])
    sx = _find_spans(ev, kx)
    sy = _find_spans(ev, ky)
    bx = _basis(ev, kx, sx).astype(np.float32)  # (1024, 4)
    by = _basis(ev, ky, sy).astype(np.float32)
    ixs = np.mod(sx[None, :] - DEGREE + np.arange(DEGREE + 1)[:, None], N_CTRL)
    iys = np.mod(sy[None, :] - DEGREE + np.arange(DEGREE + 1)[:, None], N_CTRL)

    BxD = np.zeros((OUT_XY, N_CTRL), np.float32)
    BxD[np.arange(OUT_XY)[:, None], ixs.T] = bx
    ByD = np.zeros((N_CTRL, OUT_XY), np.float32)
    ByD[iys, np.arange(OUT_XY)[None, :]] = by.T

    # Per-core stage-1 footprint windows (rows of P actually touched).
    los, widths = [], []
    for c in range(N_CORES):
        s = sx[EC * c:EC * (c + 1)]
        lo = int(s.min()) - DEGREE
        w = int(s.max()) - lo + 1
        if w > N_CTRL:  # degenerate: full wrap; use the identity window
            lo, w = 0, N_CTRL
        los.append(lo)
        widths.append(w)
    k1 = 128 if max(widths) <= 128 else N_CTRL

    # --- Banded stage 2 in unwrapped span space j' = span + l in [0, 261].
    # Each f-block of F=256 touches a single j' window of width <= ~80, so
    # one K<=128 matmul per (block, d) streams each output column exactly
    # once (the dense form streams everything twice via K=256 chunking).
    FB = 256
    NB = OUT_XY // FB
    wstarts = [int(sy[FB * b:FB * (b + 1)].min()) for b in range(NB)]
    vwidths = [int(sy[FB * b:FB * (b + 1)].max()) + DEGREE - wstarts[b] + 1
               for b in range(NB)]
    banded = (k1 == 128 and max(vwidths) <= 128
              and not bool(int(os.environ.get("NURBS_FORCE_DENSE", "0"))))
    if banded:
        # Two per-pair operand tensors so the q=0 half (blocks 0,1 + BxT)
        # lands -- and stage 1 starts -- one transfer earlier than q=1.
        # Pair q covers j' columns [wstarts[2q], wstarts[2q+1] + 128).
        base = [wstarts[0], wstarts[2]]
        wq = [wstarts[1] + 128 - base[0], wstarts[3] + 128 - base[1]]
        jj = [(base[q] + np.arange(wq[q]) - DEGREE) % N_CTRL for q in (0, 1)]
        inp1a = np.zeros((N_CORES, k1, wq[0] * 3 + EC), np.float32)
        inp1b = np.zeros((N_CORES, k1, wq[1] * 3), np.float32)
        for c in range(N_CORES):
            rows = (los[c] + np.arange(widths[c])) % N_CTRL
            inp1a[c, :widths[c], :wq[0] * 3] = \
                P[rows][:, jj[0], :].reshape(widths[c], -1)
            inp1a[c, :widths[c], wq[0] * 3:] = \
                BxD[EC * c:EC * (c + 1)][:, rows].T
            inp1b[c, :widths[c]] = \
                P[rows][:, jj[1], :].reshape(widths[c], -1)
        # packed ByD': bydp[b][r, fl] = by[f, l] at r = sy[f] + l - w_b
        bydp = np.zeros((NB, 128, FB), np.float32)
        fl = np.arange(FB)
        for b in range(NB):
            f = FB * b + fl
            for l in range(DEGREE + 1):
                bydp[b, sy[f] + l - wstarts[b], fl] = by[f, l]
        return {"mode": "banded", "inp1a": inp1a, "inp1b": inp1b,
                "bydp": bydp, "k1": k1, "wq": wq, "base": base,
                "wstarts": wstarts}

    # --- Dense fallback: [p | bxt] fused operand + full ByD.
    inp1 = np.zeros((N_CORES, k1, N_CTRL * 3 + EC), np.float32)
    for c in range(N_CORES):
        rows = (los[c] + np.arange(widths[c])) % N_CTRL
        inp1[c, :widths[c], :N_CTRL * 3] = P[rows].reshape(widths[c], -1)
        inp1[c, :widths[c], N_CTRL * 3:] = \
            BxD[EC * c:EC * (c + 1)][:, rows].T
    return {"mode": "dense", "inp1": inp1, "byd": ByD, "k1": k1}


# ----------------------------------------------------------------------------
# Device kernel
# ----------------------------------------------------------------------------

def _split_multi_waits(nc):
    """Hoist extra semaphore waits onto standalone NoOps.

    The walrus build in this container rejects any instruction carrying more
    than one SyncWait ("Too many sync wait commands"), but Tile emits the
    full wait set on the consuming instruction. Splitting them into
    preceding single-wait NoOps on the same engine stream is semantically
    identical (the engine stalls at each wait in order).
    """
    import concourse.mybir as mybir

    for fn in nc.m.functions:
        for blk in fn.blocks:
            new_insts = []
            for inst in blk.instructions:
                si = getattr(inst, "sync_info", None)
                if si is not None and si.on_wait and len(si.on_wait) > 1:
                    waits = list(si.on_wait)
                    for w in waits[:-1]:
                        new_insts.append(mybir.InstNoOp(
                            name=nc.get_next_instruction_name(),
                            sync_info=mybir.SyncInfo(on_wait=[w], on_update=[]),
                            bass_nofuse=True,
                            engine=inst.engine,
                        ))
                    inst.sync_info = mybir.SyncInfo(
                        on_wait=[waits[-1]], on_update=list(si.on_update))
                new_insts.append(inst)
            blk.instructions = new_insts
    return nc


def _strip_barriers(nc):
    """Drop Tile's entry/exit all-engine barriers (~3 us + ~4.5 us of pure
    overhead for a ~25 us kernel).

    All data dependencies in this kernel are expressed through absolute
    semaphore waits (sems start at zero on NEFF load), so the entry barrier
    only delays the fastest engine to the slowest engine's boot, and the
    exit butterfly only delays completion. The one thing the exit barrier
    does guarantee is that the final output DMAs have landed before the
    NEFF retires, so we keep explicit waits on every DMA-completion
    semaphore value that no in-kernel instruction already consumed.
    """
    import concourse.mybir as mybir

    blocks = nc.m.functions[0].blocks
    drop = (mybir.InstDrain, mybir.InstEventSemaphore, mybir.InstNoOp)
    # Entry block: remove the gather/release barrier, keep register setup.
    blocks[0].instructions = [
        i for i in blocks[0].instructions
        if not isinstance(i, (mybir.InstDrain, mybir.InstEventSemaphore))]

    # Collect semaphore updates (completion totals) and consumed waits.
    totals, waited, engines = {}, {}, {}
    for blk in blocks[:-1]:
        for inst in blk.instructions:
            si = getattr(inst, "sync_info", None)
            if si is None:
                continue
            for u in si.on_update:
                if u.update_mode in ("sem-add-imm", "sem-inc"):
                    val = u.update_value if u.update_mode == "sem-add-imm" else 1
                    totals[u.id] = totals.get(u.id, 0) + val
                    engines[u.id] = inst.engine
            for w in si.on_wait:
                if w.wait_mode == "sem-ge-imm":
                    waited[w.id] = max(waited.get(w.id, 0), w.wait_value)
    tail = blocks[-1]
    kept = [i for i in tail.instructions if not isinstance(i, drop)]
    new_tail = []
    for sem_id, total in sorted(totals.items()):
        if waited.get(sem_id, 0) < total:
            new_tail.append(mybir.InstNoOp(
                name=nc.get_next_instruction_name(),
                sync_info=mybir.SyncInfo(
                    on_wait=[mybir.SyncWait(sync_type="semaphore", id=sem_id,
                                            wait_mode="sem-ge-imm",
                                            wait_value=total)],
                    on_update=[]),
                bass_nofuse=True,
                engine=engines[sem_id],
            ))
    tail.instructions = new_tail + kept
    return nc


def _emit_warmups(nc, tc, cpool, pswpool, n):
    """PE warm-up while the input DMAs are in flight: a stream of cheap
    bf16 matmuls keeps the HAM activity window busy so the clock gate opens
    (1.2 -> 2.4 GHz) before the real matmuls arrive. Sized to end roughly
    when the first operands land (~10.5 us in)."""
    import concourse.mybir as mybir

    f32 = mybir.dt.float32
    warm_sb = cpool.tile([128, 128], mybir.dt.bfloat16, name="warm_sb")
    nc.gpsimd.memset(warm_sb[:], 0.0)
    warm_ps = pswpool.tile([128, 128], f32, name="warm_ps")
    for _ in range(n):
        nc.tensor.matmul(warm_ps[:], lhsT=warm_sb[:], rhs=warm_sb[:],
                         start=True, stop=True)


def _build_bass_banded(k1, wq, base, wstarts):
    import concourse.bass as bass
    import concourse.mybir as mybir
    from concourse.tile import TileContext

    f32 = mybir.dt.float32
    NB = len(wstarts)
    FB = OUT_XY // NB
    nc = bass.Bass()
    in1a_t = nc.dram_tensor("inp1a", [k1, wq[0] * 3 + EC], f32,
                            kind="ExternalInput")
    in1b_t = nc.dram_tensor("inp1b", [k1, wq[1] * 3], f32,
                            kind="ExternalInput")
    bydp_t = nc.dram_tensor("bydp", [NB, 128, FB], f32, kind="ExternalInput")
    out_t = nc.dram_tensor("out", [EC, 3, OUT_XY], f32, kind="ExternalOutput")

    with TileContext(nc) as tc:
        with tc.tile_pool(name="const", bufs=1) as cpool, \
             tc.tile_pool(name="ps1", bufs=3, space="PSUM") as ps1pool, \
             tc.tile_pool(name="psw", bufs=1, space="PSUM") as pswpool, \
             tc.tile_pool(name="ps2", bufs=4, space="PSUM") as ps2pool:
            _emit_warmups(nc, tc, cpool, pswpool, 25)

            # Trigger order = need order: pair-0 operands, bydp pair 0,
            # then pair-1 operands, bydp pair 1.
            in1a_sb = cpool.tile([128, wq[0] * 3 + EC], f32, name="in1a_sb")
            nc.sync.dma_start(out=in1a_sb[:], in_=in1a_t[:])
            bydp_sb = []
            t0 = cpool.tile([128, 2, FB], f32, tag="bydp0", name="bydp0")
            nc.sync.dma_start(out=t0[:],
                              in_=bydp_t[0:2].rearrange("b p f -> p b f"))
            bydp_sb.append(t0)
            in1b_sb = cpool.tile([128, wq[1] * 3], f32, name="in1b_sb")
            nc.scalar.dma_start(out=in1b_sb[:], in_=in1b_t[:])
            t1 = cpool.tile([128, 2, FB], f32, tag="bydp1", name="bydp1")
            nc.scalar.dma_start(out=t1[:],
                                in_=bydp_t[2:4].rearrange("b p f -> p b f"))
            bydp_sb.append(t1)

            # Window tiles paired (two f-blocks per tile) so stage 1 needs
            # one PSUM bank + one copy per pair instead of per block.
            t1tw_sb = [cpool.tile([128, 3, 2, EC], f32, tag=f"t1tw{q}",
                                  name=f"t1tw{q}") for q in range(NB // 2)]
            # One staging tile per output chunk: a single shared tile would
            # create false copy-after-DMA hazards (Tile tracks whole tiles).
            out_sb = {(d, q): cpool.tile([128, 2 * FB], f32,
                                         tag=f"out{d}_{q}", name=f"out{d}_{q}")
                      for d in range(3) for q in range(NB // 2)}
            pva = in1a_sb[:, :wq[0] * 3].rearrange("p (j c) -> p j c", c=3)
            pvb = in1b_sb[:, :].rearrange("p (j c) -> p j c", c=3)
            pv = [pva, pvb]
            bxt = in1a_sb[:, wq[0] * 3:]

            # Pair-major: all q=0 work only needs the first two DMAs.
            for q in range(NB // 2):
                for d in range(3):
                    # Stage 1: T1Tw[b][r,e] = sum_i Pext[i, w_b + r, d]*BxT
                    ps = ps1pool.tile([128, 2, EC], f32, tag="ps1", name="ps1")
                    for s in range(2):
                        b = 2 * q + s
                        o = wstarts[b] - base[q]
                        nc.tensor.matmul(
                            ps[:, s], lhsT=pv[q][:, o:o + 128, d],
                            rhs=bxt, start=True, stop=True)
                    if d % 2 == 0:
                        nc.vector.tensor_copy(out=t1tw_sb[q][:, d], in_=ps[:])
                    else:
                        nc.scalar.copy(out=t1tw_sb[q][:, d], in_=ps[:])
                for d in range(3):
                    # Stage 2: single K<=128 window matmul per (block, d)
                    ps2 = ps2pool.tile([128, 2, FB], f32, tag="ps2",
                                       name="ps2")
                    for s in range(2):
                        nc.tensor.matmul(
                            ps2[:, s], lhsT=t1tw_sb[q][:, d, s],
                            rhs=bydp_sb[q][:, s], start=True, stop=True)
                    dst = out_sb[(d, q)][:]
                    if q == 1 and d == 2:
                        # Final chunk: two half copies/DMAs so the first
                        # half streams while the second is still copying.
                        for s in range(2):
                            h = dst[:, s * FB:(s + 1) * FB]
                            nc.scalar.copy(out=h, in_=ps2[:, s])
                            nc.sync.dma_start(
                                out=out_t[:, d, (q * 2 + s) * FB:
                                          (q * 2 + s + 1) * FB], in_=h)
                    else:
                        if d % 2 == 0:
                            nc.vector.tensor_copy(out=dst, in_=ps2[:])
                        else:
                            nc.scalar.copy(out=dst, in_=ps2[:])
                        # Sync is idle after the input triggers; stream each
                        # 256 KB chunk out as soon as its copy lands.
                        nc.sync.dma_start(
                            out=out_t[:, d, q * 2 * FB:(q + 1) * 2 * FB],
                            in_=dst)
    return nc


def _build_bass_dense(k1):
    import concourse.bass as bass
    import concourse.mybir as mybir
    from concourse.tile import TileContext

    f32 = mybir.dt.float32
    nc = bass.Bass()
    W1 = N_CTRL * 3 + EC  # fused stage-1 operand width (896)
    in1_t = nc.dram_tensor("inp1", [k1, W1], f32, kind="ExternalInput")
    s2dt = mybir.dt.float32r if F32R else f32
    byd_in = nc.dram_tensor("byd", [N_CTRL, OUT_XY], s2dt,
                            kind="ExternalInput")
    out_t = nc.dram_tensor("out", [EC, 3, OUT_XY], f32, kind="ExternalOutput")

    nk = k1 // 128
    with TileContext(nc) as tc:
        with tc.tile_pool(name="const", bufs=1) as cpool, \
             tc.tile_pool(name="ps1", bufs=2, space="PSUM") as ps1pool, \
             tc.tile_pool(name="psw", bufs=1, space="PSUM") as pswpool, \
             tc.tile_pool(name="ps2", bufs=4, space="PSUM") as ps2pool:
            _emit_warmups(nc, tc, cpool, pswpool, 24)

            in1_sb = []
            for kc in range(nk):
                t = cpool.tile([128, W1], f32, tag=f"in1_{kc}",
                               name=f"in1_{kc}")
                nc.sync.dma_start(out=t[:], in_=in1_t[kc * 128:(kc + 1) * 128])
                in1_sb.append(t)
            byd_sb = []
            for fc in range(2):
                t = cpool.tile([128, 2, 512], s2dt, tag=f"byd{fc}",
                               name=f"byd{fc}")
                nc.sync.dma_start(
                    out=t[:],
                    in_=byd_in[:, fc * 512:(fc + 1) * 512].rearrange(
                        "(t p) f -> p t f", p=128))
                byd_sb.append(t)

            t1t_sb = [cpool.tile([128, 3, EC], s2dt, tag=f"t1t{jt}",
                                 name=f"t1t{jt}") for jt in range(2)]
            out_sb = {(d, fc): cpool.tile([128, 512], f32,
                                          tag=f"out{d}_{fc}",
                                          name=f"out{d}_{fc}")
                      for d in range(3) for fc in range(2)}

            for d in range(3):
                for jt in range(2):
                    ps = ps1pool.tile([128, EC], f32, tag="ps1", name="ps1")
                    for kc in range(nk):
                        pv = in1_sb[kc][:, :N_CTRL * 3].rearrange(
                            "p (j c) -> p j c", c=3)
                        nc.tensor.matmul(
                            ps[:],
                            lhsT=pv[:, jt * 128:(jt + 1) * 128, d],
                            rhs=in1_sb[kc][:, N_CTRL * 3:],
                            start=(kc == 0),
                            stop=(kc == nk - 1),
                        )
                    if jt == 0:
                        nc.vector.tensor_copy(out=t1t_sb[jt][:, d], in_=ps[:])
                    else:
                        nc.scalar.copy(out=t1t_sb[jt][:, d], in_=ps[:])
                for fc in range(2):
                    ps2 = ps2pool.tile([128, 512], f32, tag="ps2", name="ps2")
                    for jt in range(2):
                        nc.tensor.matmul(
                            ps2[:],
                            lhsT=t1t_sb[jt][:, d],
                            rhs=byd_sb[fc][:, jt],
                            start=(jt == 0),
                            stop=(jt == 1),
                        )
                    dst = out_sb[(d, fc)][:]
                    if fc == 0:
                        nc.vector.tensor_copy(out=dst, in_=ps2[:])
                    else:
                        nc.scalar.copy(out=dst, in_=ps2[:])
                    nc.sync.dma_start(
                        out=out_t[:, d, fc * 512:(fc + 1) * 512], in_=dst)
    return nc


def kernel(control_points, knot_vector_x, knot_vector_y):
    global last_results
    from concourse.bass_utils import run_bass_kernel_spmd

    pack = _host_pack(control_points, knot_vector_x, knot_vector_y)
    if pack["mode"] == "banded":
        nc = _build_bass_banded(pack["k1"], pack["wq"], pack["base"],
                                pack["wstarts"])
        in_maps = [{"inp1a": pack["inp1a"][c], "inp1b": pack["inp1b"][c],
                    "bydp": pack["bydp"]} for c in range(N_CORES)]
    else:
        nc = _build_bass_dense(pack["k1"])
        in_maps = [{"inp1": pack["inp1"][c], "byd": pack["byd"]}
                   for c in range(N_CORES)]
    if bool(int(os.environ.get("NURBS_STRIP", "1"))):
        nc = _strip_barriers(nc)
    nc = _split_multi_waits(nc)
    trace = bool(int(os.environ.get("NURBS_TRACE", "0")))
    res = run_bass_kernel_spmd(nc, in_maps, core_ids=list(range(N_CORES)),
                               trace=trace)
    last_results = res
    full = np.empty((1, OUT_XY, OUT_XY, 3), np.float32)
    for c in range(N_CORES):
        # per-core result is [e, d, f] -> [e, f, d]
        full[0, EC * c:EC * (c + 1)] = res.results[c]["out"].transpose(0, 2, 1)
    return full
